# revision 10
# baseline (speedup 1.0000x reference)
"""Trainium2 Bass kernel for nn_Aggregator (GNN message passing).

v4 (weights-stationary, fp8 moving xn, G=4 blocks/group) left the PE as
the bottleneck (~300ns per N=512 matmul on dense data). v5 splits the
neighbor reduction across engines:
  - PE: chunks 0..12 as accumulating matmuls (ring A data), plus the
    DVE-reduced pseudo-chunk, plus the self matmul.
  - DVE: tree-reduces chunks 13..24 (ring B data) in bf16 (11 adds,
    fp8 inputs upcast on the first level) into one [f, GP] chunk.
ACT fuses per-partition bias + relu + bf16 downcast; stores ride the
gpsimd SWDGE queue so the HWDGE rings only ever carry loads.

Numerics: fp8-e4m3 xn with bf16 tree + fp32 PSUM -> rel-to-max ~5e-3
(gate 2e-2). Traffic/core: 32.8MB xn + 2.6MB xs + 5.2MB out = 40.6MB.
"""

import sys

for _p in ("/opt/trn_rl_repo", "/root/.axon_site/_ro/trn_rl_repo"):
    if _p not in sys.path:
        sys.path.append(_p)

import numpy as np

from concourse import bacc, bass, mybir
from concourse.bass_utils import run_bass_kernel_spmd
from concourse.tile import TileContext

N_CORES = 8
B, H, NN, F = 8192, 10, 25, 128
D = 256
B_LOC = B // N_CORES          # 1024
R_LOC = B_LOC * H             # 10240 rows per core
P = 128
N_BLOCKS = R_LOC // P         # 80
G = 4                         # row-blocks per group
GP = G * P                    # 512 moving columns
N_GROUPS = N_BLOCKS // G      # 20
FP32 = mybir.dt.float32
BF16 = mybir.dt.bfloat16
FP8 = mybir.dt.float8e4
RELU = mybir.ActivationFunctionType.Relu

CN = NN * GP                  # 12800 xnt columns per group row
CW = 258                      # consts: ws, wn/25, bias_self col, bias_neigh col

PE_CHUNKS = 13                # chunks 0..12 on the PE (ring A)
DVE_CHUNKS = NN - PE_CHUNKS   # chunks 13..24 on the DVE (ring B)


def build_bass(loop_iters=None, bpt=1, xn_bufs=None, unroll_reps=1,
               xn_dtype=FP8, pe_chunks=PE_CHUNKS, psn_bufs=3, osb_bufs=3,
               pe_prered=False, dve_reduce=False):
    if xn_bufs is None:
        xn_bufs = 5
    RS = pe_chunks
    nd = NN - RS              # chunks reduced on DVE

    nc = bacc.Bacc(None)
    xst = nc.dram_tensor("xst", [N_GROUPS * F, GP], BF16, kind="ExternalInput")
    xnt = nc.dram_tensor("xnt", [N_GROUPS * F, CN], xn_dtype, kind="ExternalInput")
    consts = nc.dram_tensor("consts", [P, CW], BF16, kind="ExternalInput")
    out = nc.dram_tensor("out", [N_GROUPS * P, 2 * GP], BF16, kind="ExternalOutput")

    with TileContext(nc) as tc:
        if loop_iters is not None:
            loop_cm = tc.For_i(0, loop_iters, 1)
            loop_cm.__enter__()
        with (
            tc.tile_pool(name="const", bufs=1) as cpool,
            tc.tile_pool(name="xn", bufs=xn_bufs) as xnpool,
            tc.tile_pool(name="xs", bufs=4) as xspool,
            tc.tile_pool(name="red", bufs=4) as rpool,
            tc.tile_pool(name="osb", bufs=osb_bufs) as opool,
            tc.tile_pool(name="psS", bufs=2, space="PSUM") as pspool_s,
            tc.tile_pool(name="psN", bufs=psn_bufs, space="PSUM") as pspool_n,
        ):
            const_t = cpool.tile([P, CW], BF16)
            nc.sync.dma_start(out=const_t, in_=consts[:, :])
            ws_ap = const_t[:, 0:F]
            wn_ap = const_t[:, F : 2 * F]
            bias_s_ap = const_t[:, 2 * F : 2 * F + 1]
            bias_n_ap = const_t[:, 2 * F + 1 : 2 * F + 2]

            for _rep in range(unroll_reps):
                for g in range(N_GROUPS):
                    f0 = g * F
                    xn_t = xnpool.tile([P, CN], xn_dtype)
                    # Ring A: PE-direct chunks. Ring B: xs, then DVE chunks.
                    nc.sync.dma_start(
                        out=xn_t[:, 0 : RS * GP], in_=xnt[f0 : f0 + F, 0 : RS * GP]
                    )
                    xs_t = xspool.tile([P, GP], BF16)
                    nc.scalar.dma_start(out=xs_t, in_=xst[f0 : f0 + F, :])
                    nc.scalar.dma_start(
                        out=xn_t[:, RS * GP :], in_=xnt[f0 : f0 + F, RS * GP :]
                    )

                    # DVE reduction of chunks RS..NN-1 -> red[:, 0:GP] in 4
                    # wide strip ops (pairing is arbitrary for a sum, so wide
                    # contiguous strips replace narrow per-pair adds — same
                    # element work, far less per-instruction overhead).
                    # Level 1 adds the two strip halves (fp8 -> bf16), then
                    # contiguous halving on the bf16 partials.
                    assert nd % 2 == 0
                    npar = nd // 2

                    def ck(n):
                        return xn_t[:, n * GP : (n + 1) * GP]

                    if dve_reduce:
                        # Single-instruction reduction: requires the host to
                        # stage the DVE half column-interleaved [c, n] so the
                        # innermost reduce axis is contiguous.
                        red = rpool.tile([P, GP], BF16)
                        with nc.allow_low_precision(
                            reason="bf16 reduce output; internal accum wider"
                        ):
                            nc.vector.tensor_reduce(
                                out=red[:, 0:GP],
                                in_=xn_t[:, RS * GP :].rearrange(
                                    "p (c n) -> p c n", n=nd
                                ),
                                axis=mybir.AxisListType.X,
                                op=mybir.AluOpType.add,
                            )
                    else:
                        red = rpool.tile([P, npar * GP], BF16)
                        nc.vector.tensor_add(
                            out=red,
                            in0=xn_t[:, RS * GP : (RS + npar) * GP],
                            in1=xn_t[:, (RS + npar) * GP : NN * GP],
                        )
                        w = npar
                        while w > 1:
                            h = w // 2
                            nc.vector.tensor_add(
                                out=red[:, 0 : h * GP],
                                in0=red[:, 0 : h * GP],
                                in1=red[:, (w - h) * GP : w * GP],
                            )
                            w -= h

                    self_ps = pspool_s.tile([P, GP], FP32)
                    neigh_ps = pspool_n.tile([P, GP], FP32)

                    if pe_prered:
                        # One extra DVE level over the PE-side chunks too:
                        # chunks 0..11 -> 6 bf16 partials, so the PE runs
                        # 6 partial MMs + chunk 12 + the DVE-tree chunk
                        # (9 MMs total with self) — fewer MACs + fewer
                        # redundant LDWEIGHTS at modest DVE cost.
                        assert RS == 13
                        red2 = rpool.tile([P, 6 * GP], BF16)
                        nc.vector.tensor_add(
                            out=red2,
                            in0=xn_t[:, 0 : 6 * GP],
                            in1=xn_t[:, 6 * GP : 12 * GP],
                        )
                        pe_rhs = [red2[:, k * GP : (k + 1) * GP] for k in range(6)]
                        pe_rhs.append(ck(12))
                    else:
                        pe_rhs = [ck(n) for n in range(RS)]
                    for i, rhs in enumerate(pe_rhs):
                        nc.tensor.matmul(
                            out=neigh_ps,
                            lhsT=wn_ap,
                            rhs=rhs,
                            start=(i == 0), stop=False, skip_group_check=True,
                        )
                    nc.tensor.matmul(
                        out=neigh_ps, lhsT=wn_ap, rhs=red[:, 0:GP],
                        start=False, stop=True, skip_group_check=True,
                    )
                    nc.tensor.matmul(
                        out=self_ps, lhsT=ws_ap, rhs=xs_t,
                        start=True, stop=True, skip_group_check=True,
                    )

                    o_sb = opool.tile([P, 2 * GP], BF16)
                    nc.scalar.activation(
                        out=o_sb[:, 0:GP], in_=self_ps, func=RELU, bias=bias_s_ap
                    )
                    nc.scalar.activation(
                        out=o_sb[:, GP : 2 * GP], in_=neigh_ps, func=RELU,
                        bias=bias_n_ap,
                    )
                    # Stores alone on the gpsimd SWDGE queue.
                    nc.gpsimd.dma_start(
                        out=out[g * P : (g + 1) * P, :], in_=o_sb
                    )

        if loop_iters is not None:
            loop_cm.__exit__(None, None, None)

    nc.compile()
    return nc


_NC_CACHE = None


def kernel(x_self, x_neigh, w_neigh, w_self, bias):
    import ml_dtypes

    global _NC_CACHE
    if _NC_CACHE is None:
        _NC_CACHE = build_bass()
    nc = _NC_CACHE

    ng = N_CORES * N_GROUPS
    xn8 = np.asarray(x_neigh).astype(ml_dtypes.float8_e4m3)
    xn8 = xn8.reshape(ng, G, P, NN, F)
    xnt = np.ascontiguousarray(xn8.transpose(0, 4, 3, 1, 2)).reshape(ng * F, CN)
    xsb = np.asarray(x_self).astype(ml_dtypes.bfloat16).reshape(ng, G, P, F)
    xst = np.ascontiguousarray(xsb.transpose(0, 3, 1, 2)).reshape(ng * F, GP)

    consts = np.zeros((P, CW), dtype=np.float32)
    consts[:, 0:F] = np.asarray(w_self, dtype=np.float32)
    consts[:, F : 2 * F] = np.asarray(w_neigh, dtype=np.float32) / np.float32(NN)
    consts[:, 2 * F] = np.asarray(bias, dtype=np.float32)[0:P]
    consts[:, 2 * F + 1] = np.asarray(bias, dtype=np.float32)[P:D]
    consts = consts.astype(ml_dtypes.bfloat16)

    rg = N_GROUPS * F
    in_maps = [
        {"xst": xst[c * rg : (c + 1) * rg], "xnt": xnt[c * rg : (c + 1) * rg],
         "consts": consts}
        for c in range(N_CORES)
    ]

    res = run_bass_kernel_spmd(nc, in_maps, list(range(N_CORES)))
    out = np.concatenate([res.results[c]["out"] for c in range(N_CORES)], axis=0)
    # out[g*P + d, h*GP + j*P + r] -> full[(g*G+j)*P + r, h*P + d]
    o = out.astype(np.float32).reshape(ng, P, 2, G, P)
    o = o.transpose(0, 3, 4, 2, 1).reshape(B, H, D)
    return o


# revision 11
# speedup vs baseline: 1.0424x; 1.0424x over previous
"""Trainium2 Bass kernel for nn_Aggregator (GNN message passing).

v4 (weights-stationary, fp8 moving xn, G=4 blocks/group) left the PE as
the bottleneck (~300ns per N=512 matmul on dense data). v5 splits the
neighbor reduction across engines:
  - PE: chunks 0..12 as accumulating matmuls (ring A data), plus the
    DVE-reduced pseudo-chunk, plus the self matmul.
  - DVE: tree-reduces chunks 13..24 (ring B data) in bf16 (11 adds,
    fp8 inputs upcast on the first level) into one [f, GP] chunk.
ACT fuses per-partition bias + relu + bf16 downcast; stores ride the
gpsimd SWDGE queue so the HWDGE rings only ever carry loads.

Numerics: fp8-e4m3 xn with bf16 tree + fp32 PSUM -> rel-to-max ~5e-3
(gate 2e-2). Traffic/core: 32.8MB xn + 2.6MB xs + 5.2MB out = 40.6MB.
"""

import sys

for _p in ("/opt/trn_rl_repo", "/root/.axon_site/_ro/trn_rl_repo"):
    if _p not in sys.path:
        sys.path.append(_p)

import numpy as np

from concourse import bacc, bass, mybir
from concourse.bass_utils import run_bass_kernel_spmd
from concourse.tile import TileContext

N_CORES = 8
B, H, NN, F = 8192, 10, 25, 128
D = 256
B_LOC = B // N_CORES          # 1024
R_LOC = B_LOC * H             # 10240 rows per core
P = 128
N_BLOCKS = R_LOC // P         # 80
G = 4                         # row-blocks per group
GP = G * P                    # 512 moving columns
N_GROUPS = N_BLOCKS // G      # 20
FP32 = mybir.dt.float32
BF16 = mybir.dt.bfloat16
FP8 = mybir.dt.float8e4
RELU = mybir.ActivationFunctionType.Relu

CN = NN * GP                  # 12800 xnt columns per group row
CW = 258                      # consts: ws, wn/25, bias_self col, bias_neigh col

PE_CHUNKS = 13                # chunks 0..12 on the PE (ring A)
DVE_CHUNKS = NN - PE_CHUNKS   # chunks 13..24 on the DVE (ring B)


def build_bass(loop_iters=None, bpt=1, xn_bufs=None, unroll_reps=1,
               xn_dtype=FP8, pe_chunks=PE_CHUNKS, psn_bufs=3, osb_bufs=3,
               pe_prered=False, dve_reduce=False):
    if xn_bufs is None:
        xn_bufs = 5
    RS = pe_chunks
    nd = NN - RS              # chunks reduced on DVE

    nc = bacc.Bacc(None)
    xst = nc.dram_tensor("xst", [N_GROUPS * F, GP], BF16, kind="ExternalInput")
    xnt = nc.dram_tensor("xnt", [N_GROUPS * F, CN], xn_dtype, kind="ExternalInput")
    consts = nc.dram_tensor("consts", [P, CW], BF16, kind="ExternalInput")
    out = nc.dram_tensor("out", [N_GROUPS * P, 2 * GP], BF16, kind="ExternalOutput")

    with TileContext(nc) as tc:
        with (
            tc.tile_pool(name="const", bufs=1) as cpool,
            tc.tile_pool(name="xn", bufs=xn_bufs) as xnpool,
            tc.tile_pool(name="xs", bufs=4) as xspool,
            tc.tile_pool(name="red", bufs=4) as rpool,
            tc.tile_pool(name="osb", bufs=osb_bufs) as opool,
            tc.tile_pool(name="psS", bufs=2, space="PSUM") as pspool_s,
            tc.tile_pool(name="psN", bufs=psn_bufs, space="PSUM") as pspool_n,
        ):
            const_t = cpool.tile([P, CW], BF16)
            nc.sync.dma_start(out=const_t, in_=consts[:, :])
            ws_ap = const_t[:, 0:F]
            wn_ap = const_t[:, F : 2 * F]
            bias_s_ap = const_t[:, 2 * F : 2 * F + 1]
            bias_n_ap = const_t[:, 2 * F + 1 : 2 * F + 2]

            # The const load stays OUTSIDE the hardware loop: reloading it
            # per iteration puts a WAR-blocked DMA at the ring A queue head
            # and drains the whole pipeline at every loop boundary.
            if loop_iters is not None:
                loop_cm = tc.For_i(0, loop_iters, 1)
                loop_cm.__enter__()
            for _rep in range(unroll_reps):
                for g in range(N_GROUPS):
                    f0 = g * F
                    xn_t = xnpool.tile([P, CN], xn_dtype)
                    # Ring A: PE-direct chunks. Ring B: xs, then DVE chunks.
                    nc.sync.dma_start(
                        out=xn_t[:, 0 : RS * GP], in_=xnt[f0 : f0 + F, 0 : RS * GP]
                    )
                    xs_t = xspool.tile([P, GP], BF16)
                    nc.scalar.dma_start(out=xs_t, in_=xst[f0 : f0 + F, :])
                    nc.scalar.dma_start(
                        out=xn_t[:, RS * GP :], in_=xnt[f0 : f0 + F, RS * GP :]
                    )

                    # DVE reduction of chunks RS..NN-1 -> red[:, 0:GP] in 4
                    # wide strip ops (pairing is arbitrary for a sum, so wide
                    # contiguous strips replace narrow per-pair adds — same
                    # element work, far less per-instruction overhead).
                    # Level 1 adds the two strip halves (fp8 -> bf16), then
                    # contiguous halving on the bf16 partials.
                    assert nd % 2 == 0
                    npar = nd // 2

                    def ck(n):
                        return xn_t[:, n * GP : (n + 1) * GP]

                    if dve_reduce:
                        # Single-instruction reduction: requires the host to
                        # stage the DVE half column-interleaved [c, n] so the
                        # innermost reduce axis is contiguous.
                        red = rpool.tile([P, GP], BF16)
                        with nc.allow_low_precision(
                            reason="bf16 reduce output; internal accum wider"
                        ):
                            nc.vector.tensor_reduce(
                                out=red[:, 0:GP],
                                in_=xn_t[:, RS * GP :].rearrange(
                                    "p (c n) -> p c n", n=nd
                                ),
                                axis=mybir.AxisListType.X,
                                op=mybir.AluOpType.add,
                            )
                    else:
                        red = rpool.tile([P, npar * GP], BF16)
                        nc.vector.tensor_add(
                            out=red,
                            in0=xn_t[:, RS * GP : (RS + npar) * GP],
                            in1=xn_t[:, (RS + npar) * GP : NN * GP],
                        )
                        w = npar
                        while w > 1:
                            h = w // 2
                            nc.vector.tensor_add(
                                out=red[:, 0 : h * GP],
                                in0=red[:, 0 : h * GP],
                                in1=red[:, (w - h) * GP : w * GP],
                            )
                            w -= h

                    self_ps = pspool_s.tile([P, GP], FP32)
                    neigh_ps = pspool_n.tile([P, GP], FP32)

                    if pe_prered:
                        # One extra DVE level over the PE-side chunks too:
                        # chunks 0..11 -> 6 bf16 partials, so the PE runs
                        # 6 partial MMs + chunk 12 + the DVE-tree chunk
                        # (9 MMs total with self) — fewer MACs + fewer
                        # redundant LDWEIGHTS at modest DVE cost.
                        assert RS == 13
                        red2 = rpool.tile([P, 6 * GP], BF16)
                        nc.vector.tensor_add(
                            out=red2,
                            in0=xn_t[:, 0 : 6 * GP],
                            in1=xn_t[:, 6 * GP : 12 * GP],
                        )
                        pe_rhs = [red2[:, k * GP : (k + 1) * GP] for k in range(6)]
                        pe_rhs.append(ck(12))
                    else:
                        pe_rhs = [ck(n) for n in range(RS)]
                    for i, rhs in enumerate(pe_rhs):
                        nc.tensor.matmul(
                            out=neigh_ps,
                            lhsT=wn_ap,
                            rhs=rhs,
                            start=(i == 0), stop=False, skip_group_check=True,
                        )
                    nc.tensor.matmul(
                        out=neigh_ps, lhsT=wn_ap, rhs=red[:, 0:GP],
                        start=False, stop=True, skip_group_check=True,
                    )
                    nc.tensor.matmul(
                        out=self_ps, lhsT=ws_ap, rhs=xs_t,
                        start=True, stop=True, skip_group_check=True,
                    )

                    o_sb = opool.tile([P, 2 * GP], BF16)
                    nc.scalar.activation(
                        out=o_sb[:, 0:GP], in_=self_ps, func=RELU, bias=bias_s_ap
                    )
                    nc.scalar.activation(
                        out=o_sb[:, GP : 2 * GP], in_=neigh_ps, func=RELU,
                        bias=bias_n_ap,
                    )
                    # Stores alone on the gpsimd SWDGE queue.
                    nc.gpsimd.dma_start(
                        out=out[g * P : (g + 1) * P, :], in_=o_sb
                    )

        if loop_iters is not None:
            loop_cm.__exit__(None, None, None)

    nc.compile()
    return nc


_NC_CACHE = None


def kernel(x_self, x_neigh, w_neigh, w_self, bias):
    import ml_dtypes

    global _NC_CACHE
    if _NC_CACHE is None:
        _NC_CACHE = build_bass()
    nc = _NC_CACHE

    ng = N_CORES * N_GROUPS
    xn8 = np.asarray(x_neigh).astype(ml_dtypes.float8_e4m3)
    xn8 = xn8.reshape(ng, G, P, NN, F)
    xnt = np.ascontiguousarray(xn8.transpose(0, 4, 3, 1, 2)).reshape(ng * F, CN)
    xsb = np.asarray(x_self).astype(ml_dtypes.bfloat16).reshape(ng, G, P, F)
    xst = np.ascontiguousarray(xsb.transpose(0, 3, 1, 2)).reshape(ng * F, GP)

    consts = np.zeros((P, CW), dtype=np.float32)
    consts[:, 0:F] = np.asarray(w_self, dtype=np.float32)
    consts[:, F : 2 * F] = np.asarray(w_neigh, dtype=np.float32) / np.float32(NN)
    consts[:, 2 * F] = np.asarray(bias, dtype=np.float32)[0:P]
    consts[:, 2 * F + 1] = np.asarray(bias, dtype=np.float32)[P:D]
    consts = consts.astype(ml_dtypes.bfloat16)

    rg = N_GROUPS * F
    in_maps = [
        {"xst": xst[c * rg : (c + 1) * rg], "xnt": xnt[c * rg : (c + 1) * rg],
         "consts": consts}
        for c in range(N_CORES)
    ]

    res = run_bass_kernel_spmd(nc, in_maps, list(range(N_CORES)))
    out = np.concatenate([res.results[c]["out"] for c in range(N_CORES)], axis=0)
    # out[g*P + d, h*GP + j*P + r] -> full[(g*G+j)*P + r, h*P + d]
    o = out.astype(np.float32).reshape(ng, P, 2, G, P)
    o = o.transpose(0, 3, 4, 2, 1).reshape(B, H, D)
    return o
